# revision 77
# baseline (speedup 1.0000x reference)
"""GCN encoder (2-layer GCNConv) on 8 Trainium2 NeuronCores.

Strategy (pull model, dst-sharded, host-routed halo):
  out = A @ relu(A @ x @ W1 + b1) @ W2 + b2,  A = D^-1/2 (Adj+I) D^-1/2
Reassociate: agg = A @ x first, then dense matmul by W (A@(xW) == (A@x)W).
Fold the src-side dinv into the node table on the host (x~ = dinv * x) and
the dst-side dinv into a per-partition ACT scale.

The per-edge gather (the SWDGE descriptor-generation bottleneck of the
first version: GpSimd was 90% busy emitting one descriptor per edge) is
done ON THE HOST: the edge list is known at preprocessing time and the
node table passes through the host anyway (x is an input; h1 must make a
host roundtrip because the axon terminal cannot run collectives), so the
host materializes each core's edge messages x~[src[e]] directly in the
PE-chunk layout.  The device then just streams contiguous bf16 blocks
(one fat descriptor per partition per group DMA, spread over all 16 DMA
engines) — pure HBM bandwidth, no gather.

Aggregation uses 64-wide dst tiles: chunks of 128 edges feed PE
selection-matrix matmuls (self loops folded in as ordinary edges):
  psum[feat128, dst64] += M_chunk[e, feat].T @ S_chunk[e, dst64]
The 64-wide tiles halve the S-build work on DVE (the is_equal runs at
1 elem/lane/cycle due to the broadcast dstp operand) and shorten each
matmul; S is built once per group of 7 tiles in a single batched
is_equal.  Outputs are written in a [64, tiles*odim] partition-major
DRAM layout (one descriptor per partition) and unpermuted on the host.
"""

import os

import numpy as np
import ml_dtypes

# ---------------------------------------------------------------- constants
N_NODES = 100000
N_EDGES = 1600000
IN_DIM = 128
HID_DIM = 128
OUT_DIM = 64
P = 128                     # edge-chunk size (PE contraction dim)
W = 64                      # dst-tile width

N_CORES = 8
TPC = 196                   # tiles per core
SPC = TPC * W               # 12544 slots per core
NS = N_CORES * SPC          # 100352 slots total
NT = N_CORES * TPC          # 1568 tiles total
GROUP_TILES = 7
N_GROUPS = TPC // GROUP_TILES   # 28

BF16 = ml_dtypes.bfloat16
SENT_ROW = NS               # table_ext[NS] is an all-zero row
SENT_DST = 200.0            # dst-local sentinel: matches no iota value

LAST_RESULTS = None


# ================================================================ host prep
def _preprocess(x, edge_index):
    x = np.asarray(x, dtype=np.float32)
    ei = np.asarray(edge_index, dtype=np.int64)
    src = ei[0]
    dst = ei[1]

    # degree includes the self loop (appended by the reference)
    deg = (np.bincount(dst, minlength=N_NODES) + 1).astype(np.float64)
    dinv = 1.0 / np.sqrt(np.maximum(deg, 1e-12))

    # ---- slot assignment: band packing.  Tiles 0..N9-1 target 9 chunks
    # (<=1152 edge rows incl self), the rest 8 (<=1024), so chunk counts
    # hug ceil(rows/128) with ~0 padding instead of the ~6% a uniform
    # degree spread gives.  Nodes are snake-dealt to cores, LPT-split into
    # the two bands per core, and snake-dealt within each band.
    rows_of = deg.astype(np.int64)            # deg+1 per node... deg incl self
    order = np.argsort(-rows_of, kind="stable")
    idx = np.arange(N_NODES)
    r8 = idx // N_CORES
    p8 = idx % N_CORES
    core_of_rank = np.where(r8 % 2 == 0, p8, N_CORES - 1 - p8)

    N9 = 104                                  # tiles in the 9-chunk band
    N8 = TPC - N9
    T9_target = N9 * (9 * P - 5)              # pace 1147: slack vs both caps
    slot_of_node = np.empty(N_NODES, dtype=np.int64)
    for c in range(N_CORES):
        nodes = order[core_of_rank == c]      # this core's nodes, deg desc
        rows = rows_of[nodes]
        npad = SPC - len(nodes)               # empty slots: 1 self row each
        rows_all = np.concatenate([rows, np.ones(npad, dtype=np.int64)])
        total = int(rows_all.sum())
        # LPT split into band9 / band8 by remaining pace
        band = np.empty(len(rows_all), dtype=np.int8)
        rem9, s9 = float(min(T9_target, total - N8 * W)), N9 * W
        rem8, s8 = float(total) - rem9, N8 * W
        for i, rv in enumerate(rows_all):
            if s9 > 0 and (s8 == 0 or rem9 * s8 >= rem8 * s9):
                band[i] = 0
                rem9 -= rv
                s9 -= 1
            else:
                band[i] = 1
                rem8 -= rv
                s8 -= 1
        # snake within each band over its tiles
        for bid, t0, ntl in ((0, 0, N9), (1, N9, N8)):
            sel = np.nonzero(band == bid)[0]
            sel = sel[sel < len(nodes)]       # drop empty-slot dummies
            kk = np.arange(len(sel))
            rr = kk // ntl
            pp = kk % ntl
            tl = t0 + np.where(rr % 2 == 0, pp, ntl - 1 - pp)
            slot_of_node[nodes[sel]] = (c * TPC + tl) * W + rr

    # ---- per-edge quantities (self loops folded in as ordinary edges)
    all_dst = np.concatenate([slot_of_node[dst], np.arange(NS, dtype=np.int64)])
    all_src = np.concatenate([slot_of_node[src], np.arange(NS, dtype=np.int64)])
    gtile = all_dst // W                      # global tile id
    dstl = (all_dst % W).astype(np.int32)

    order_e = np.argsort(gtile, kind="stable")
    gt_sorted = gtile[order_e]
    seg = np.searchsorted(gt_sorted, np.arange(NT + 1))
    seg_len = np.diff(seg)                    # edges (incl self) per gtile

    # shared static geometry: chunks per local tile = max over cores
    per_core_len = seg_len.reshape(N_CORES, TPC)
    k_tl = (-(-per_core_len.max(axis=0) // P)).astype(np.int64)   # [TPC]
    chunk_off = np.concatenate([[0], np.cumsum(k_tl)])
    C_TOT = int(chunk_off[-1])
    NCH_MAX = int(k_tl.max())
    # variable-size groups: two small warm-up groups so the pipeline
    # fills quickly, then steady 7-tile groups
    gtiles = [2, 5] + [GROUP_TILES] * ((TPC - 7) // GROUP_TILES)
    assert sum(gtiles) == TPC
    gbounds = np.concatenate([[0], np.cumsum(gtiles)])
    gco = [int(chunk_off[b]) for b in gbounds]
    NG = len(gtiles)
    C_GRP_MAX = max(gco[g + 1] - gco[g] for g in range(NG))

    # ---- per-core fill of srcrows (gather plan) and dstp (dst-local ids)
    rank = np.arange(len(order_e), dtype=np.int64) - seg[gt_sorted]
    tl_sorted = gt_sorted % TPC
    core_sorted = gt_sorted // TPC
    col_local = chunk_off[tl_sorted] + rank // P          # chunk col in core
    e_local = rank % P

    srcrows = np.full((N_CORES, C_TOT * P), SENT_ROW, dtype=np.int64)
    srcrows[core_sorted, col_local * P + e_local] = all_src[order_e]
    dstp = np.full((N_CORES, P, C_TOT), SENT_DST, dtype=BF16)
    dstp[core_sorted, e_local, col_local] = dstl[order_e].astype(BF16)



    # ---- per-core dinv (ACT scale) and rdinv (bias rank-1 lhsT)
    dinv_slots = np.zeros(NS, dtype=np.float32)
    dinv_slots[slot_of_node] = dinv.astype(np.float32)
    rdinv_slots = np.zeros(NS, dtype=np.float32)
    rdinv_slots[slot_of_node] = (1.0 / dinv).astype(np.float32)
    dinv_row = dinv_slots.reshape(N_CORES, 1, SPC)                          # [c,1,12544]
    # layer-1 scale is dinv^2: it also folds the src-side dinv the
    # layer-2 table needs into h1 (relu commutes with scale>0)
    dinv2_row = (dinv_row * dinv_row).astype(np.float32)
    rdinv_row = rdinv_slots.reshape(N_CORES, 1, SPC).astype(BF16)           # [c,1,12544]

    # imat[e, c, d] = d  (constant is_equal operand, batched per group)
    imat = np.broadcast_to(
        np.arange(W, dtype=np.float32).astype(BF16)[None, None, :],
        (P, C_GRP_MAX, W)).reshape(P, C_GRP_MAX * W).copy()

    # ---- node-feature table in slot order, pre-scaled by dinv (bf16)
    xt = np.zeros((NS, IN_DIM), dtype=BF16)
    xt[slot_of_node] = (x * dinv[:, None].astype(np.float32)).astype(BF16)

    geom = dict(
        k_tl=k_tl, chunk_off=chunk_off, C_TOT=C_TOT,
        NCH_MAX=NCH_MAX, gco=gco, C_GRP_MAX=C_GRP_MAX,
        gbounds=gbounds, NG=NG,
    )
    return dict(
        srcrows=srcrows, dstp=dstp, xt=xt, imat=imat,
        dinv_row=dinv_row, dinv2_row=dinv2_row, rdinv_row=rdinv_row,
        slot_of_node=slot_of_node, geom=geom,
    )


def _expand(table, srcrows_c, fdim):
    """Host-side halo routing: materialize per-edge messages in PE-chunk
    layout [128 e, C_TOT, fdim] bf16 from the slot table (+ zero pad row)."""
    table_ext = np.vstack([table, np.zeros((1, fdim), dtype=table.dtype)])
    rows = table_ext[srcrows_c]                       # [C_TOT*128, fdim]
    ctot = rows.shape[0] // P
    return np.ascontiguousarray(
        rows.reshape(ctot, P, fdim).transpose(1, 0, 2))


def _unpermute(out_c, odim, layer):
    """Device layout -> [SPC, odim] slot-major."""
    if layer == 1:                            # [odim, SPC]
        return np.ascontiguousarray(out_c.T)
    # layer 2: [W, TPC*odim]
    return np.ascontiguousarray(
        out_c.reshape(W, TPC, odim).transpose(1, 0, 2).reshape(SPC, odim))


# ============================================================ numpy emulator
def _emulate(prep, W1, b1, W2, b2):
    """Fast numpy mirror of the device kernel (fp32 math on bf16-rounded
    data) to validate the host-side layout before burning a HW run."""
    geom = prep["geom"]
    C_TOT = geom["C_TOT"]
    k_tl = geom["k_tl"]
    chunk_off = geom["chunk_off"]
    rdinv = prep["rdinv_row"].astype(np.float32)

    # dst slot (core-local) of every mexp position, sentinel -1
    dst_of_pos = np.full((N_CORES, C_TOT * P), -1, dtype=np.int64)
    for c in range(N_CORES):
        d = prep["dstp"][c].astype(np.float32)        # [128, C_TOT]
        for tl in range(TPC):
            for j in range(int(k_tl[tl])):
                col = chunk_off[tl] + j
                dloc = d[:, col]
                valid = dloc < W
                dst_of_pos[c, (col * P + np.arange(P))[valid]] = \
                    tl * W + dloc[valid].astype(np.int64)

    def layer(table, w, bvec, relu, odim, scale_t, sc_bf16=True):
        out = np.zeros((N_CORES, SPC, table.shape[1]), dtype=np.float32)
        for c in range(N_CORES):
            rows = _expand(table, prep["srcrows"][c], table.shape[1])
            rows = rows.transpose(1, 0, 2).reshape(C_TOT * P, -1).astype(np.float32)
            dpos = dst_of_pos[c]
            valid = dpos >= 0
            o = np.argsort(dpos[valid], kind="stable")
            rv = rows[valid][o]
            dv = dpos[valid][o]
            agg = np.zeros((SPC + 1, table.shape[1]), dtype=np.float32)
            uniq = np.unique(dv)
            red = np.add.reduceat(rv, np.searchsorted(dv, uniq), axis=0)
            agg[uniq] = red
            out[c] = agg[:SPC]
        res = np.zeros((N_CORES, SPC, odim), dtype=np.float32)
        for c in range(N_CORES):
            aggT = out[c].astype(BF16).astype(np.float32)
            ps = aggT if w is None else aggT @ w
            ps = ps + rdinv[c, 0][:, None] * bvec[None, :]
            if relu:
                ps = np.maximum(ps, 0.0)
            sc = scale_t[c, 0]
            if sc_bf16:
                sc = sc.astype(BF16)
            res[c] = ps * sc.astype(np.float32)[:, None]
        return res

    w1 = np.asarray(W1, np.float32).astype(BF16).astype(np.float32)
    w2 = np.asarray(W2, np.float32).astype(BF16).astype(np.float32)
    b1f = np.asarray(b1, np.float32).astype(BF16).astype(np.float32)
    b2f = np.asarray(b2, np.float32).astype(BF16).astype(np.float32)
    h1 = layer(prep["xt"], w1, b1f, True, HID_DIM, prep["dinv2_row"])
    h1t = h1.reshape(NS, HID_DIM).astype(BF16).astype(np.float32)
    # transform-first: v = h1~ @ W2 happens at the tail of the layer-1 NEFF
    vt = (h1t @ w2).astype(BF16)
    out = layer(vt, None, b2f, False, OUT_DIM, prep["dinv_row"], sc_bf16=False)
    return out.reshape(NS, OUT_DIM)[prep["slot_of_node"]]


# ============================================================= bass kernel
# The axon terminal cannot run ncfw collectives (NRT_EXEC_UNIT_UNRECOVERABLE),
# so the two GCN layers run as two NEFFs with a host-side h1 exchange; the
# host also routes the per-edge halo (expanded message tables) for each NEFF.
_CACHED = {}


def _build_layer_nc(layer, geom, has_bias):
    gkey = (geom["C_TOT"], hash(geom["k_tl"].tobytes()))
    key = (layer, gkey, has_bias)
    if key in _CACHED:
        return _CACHED[key]

    import concourse.mybir as mybir
    import concourse.tile as tile
    from concourse import bacc

    f32 = mybir.dt.float32
    bf16 = mybir.dt.bfloat16

    # layer 1: 128-wide messages, psW = W1.T@agg (+bias), relu, x dinv^2,
    #          then transform-first tail v = (.)@W2 -> bf16 [64, SPC] out.
    # layer 2: 64-wide pre-transformed messages, agg IS the output
    #          (+rank-1 bias in psum), x dinv -> f32 [64, SPC] out.
    fdim = IN_DIM if layer == 1 else OUT_DIM
    hdim = HID_DIM                      # layer-1 hidden width
    odim = OUT_DIM
    relu = layer == 1

    nc = bacc.Bacc("TRN2", target_bir_lowering=False, debug=False,
                   num_devices=N_CORES, name=f"gcnx_l{layer}")

    k_tl = geom["k_tl"]
    chunk_off = geom["chunk_off"]
    C_TOT = geom["C_TOT"]
    gco = geom["gco"]
    C_GRP_MAX = geom["C_GRP_MAX"]
    gbounds = geom["gbounds"]
    NG = geom["NG"]

    GW = GROUP_TILES * W

    mexp_d = nc.dram_tensor("mexp", [P, C_TOT * fdim], bf16, kind="ExternalInput")
    dstp_d = nc.dram_tensor("dstp", [P, C_TOT], bf16, kind="ExternalInput")
    imat_d = nc.dram_tensor("imat", [P, C_GRP_MAX * W], bf16,
                            kind="ExternalInput")
    if layer == 1:
        dsc_d = nc.dram_tensor("dsc", [1, SPC], bf16, kind="ExternalInput")
        w_d = nc.dram_tensor("w", [IN_DIM, hdim], bf16, kind="ExternalInput")
        w2_d = nc.dram_tensor("w2", [hdim, odim], bf16, kind="ExternalInput")
        b_d = nc.dram_tensor("b", [1, hdim], bf16, kind="ExternalInput")
        rdinv_d = nc.dram_tensor("rdinv", [1, SPC], bf16, kind="ExternalInput")
        out_d = nc.dram_tensor("out", [odim, SPC], bf16, kind="ExternalOutput")
    else:
        # transposed agg psum [dst, vfeat]: the dinv scale rides the ACT
        # copy's per-partition scale; bias (when present) rides the psum.
        dinvt_d = nc.dram_tensor("dinvt", [W, TPC], f32, kind="ExternalInput")
        b_d = nc.dram_tensor("b", [1, odim], bf16, kind="ExternalInput")
        rdinv_d = nc.dram_tensor("rdinv", [1, SPC], bf16, kind="ExternalInput")
        out_d = nc.dram_tensor("out", [W, TPC * odim], f32, kind="ExternalOutput")

    with tile.TileContext(nc) as tc:
        with (
            tc.tile_pool(name="const", bufs=1) as constp,
            tc.tile_pool(name="mbuf", bufs=5) as mpool,
            tc.tile_pool(name="gpool", bufs=3) as gpool,
            tc.tile_pool(name="sbuf_s", bufs=4) as spool,
            tc.tile_pool(name="agg", bufs=3) as aggp,
            tc.tile_pool(name="hbuf", bufs=3) as hpool,
            tc.tile_pool(name="outp", bufs=3) as outp,
            tc.tile_pool(name="psA", bufs=4, space="PSUM") as psA,
            tc.tile_pool(name="psB", bufs=2, space="PSUM") as psB,
            tc.tile_pool(name="psV", bufs=1, space="PSUM") as psV,
            tc.tile_pool(name="psR", bufs=1, space="PSUM") as psR,
        ):
            def load_group(g):
                ncols = gco[g + 1] - gco[g]
                m_t = mpool.tile([P, C_GRP_MAX, fdim], bf16, tag="m")
                nc.sync.dma_start(
                    m_t[:, :ncols, :],
                    mexp_d[:, gco[g] * fdim:gco[g + 1] * fdim]
                    .rearrange("p (c f) -> p c f", f=fdim))
                return m_t

            # is_equal operands first so S-building starts immediately,
            # then group prefetch, then the remaining constants
            dstp_sb = constp.tile([P, C_TOT], bf16)
            nc.sync.dma_start(dstp_sb[:], dstp_d[:, :])
            imat_sb = constp.tile([P, C_GRP_MAX, W], bf16)
            nc.sync.dma_start(
                imat_sb[:],
                imat_d[:, :].rearrange("p (c d) -> p c d", d=W))
            m_tiles = [load_group(0), load_group(1), load_group(2)]
            rdinv_sb = constp.tile([1, SPC], bf16)
            nc.sync.dma_start(rdinv_sb[:], rdinv_d[:, :])
            if layer == 1:
                w_sb = constp.tile([IN_DIM, hdim], bf16)
                nc.sync.dma_start(w_sb[:], w_d[:, :])
                w2_sb = constp.tile([hdim, odim], bf16)
                nc.sync.dma_start(w2_sb[:], w2_d[:, :])
                dsc_sb = constp.tile([1, SPC], bf16)
                nc.sync.dma_start(dsc_sb[:], dsc_d[:, :])
                ones_sb = constp.tile([1, odim], bf16)
                nc.gpsimd.memset(ones_sb[:], 1.0)
                b_sb = constp.tile([1, hdim], bf16)
            else:
                dinvt_sb = constp.tile([W, TPC], f32)
                nc.sync.dma_start(dinvt_sb[:], dinvt_d[:, :])
                b_sb = constp.tile([1, odim], bf16)
            nc.sync.dma_start(b_sb[:], b_d[:, :])

            def build_s(g):
                # S[e, c, d] = (dstp[e, c] == d) for the whole group at
                # once.  PE strided operands are catastrophic in either
                # slot (rhs 227ns/MM, lhsT similar), so S must stay
                # [e, c, d]-contiguous and the is_equal runs 1x.
                gc0 = gco[g]
                ncols = gco[g + 1] - gc0
                s_g = spool.tile([P, C_GRP_MAX, W], bf16, tag="s")
                nc.vector.tensor_tensor(
                    s_g[:, :ncols, :],
                    dstp_sb[:, gc0:gc0 + ncols]
                    .unsqueeze(2).broadcast_to([P, ncols, W]),
                    imat_sb[:, :ncols, :],
                    mybir.AluOpType.is_equal,
                )
                return s_g

            # S-builds primed 2 groups ahead: the DVE is strict FIFO, so
            # the late-dependency stt of group g must not sit in front of
            # the is_equal needed by group g+2
            s_tiles = [build_s(0), build_s(1)]

            def emit_tail(t0, ntl, gw, agg_g, dscr_g):
                agg_flat = agg_g[:, :ntl, :].rearrange("p t d -> p (t d)")
                ps_w = psB.tile([hdim, GW], f32, tag="psw")
                if has_bias:
                    nc.tensor.matmul(
                        ps_w[:, :gw], lhsT=b_sb[:],
                        rhs=rdinv_sb[:, t0 * W:t0 * W + gw],
                        start=True, stop=False)
                nc.tensor.matmul(
                    ps_w[:, :gw], lhsT=w_sb[:], rhs=agg_flat,
                    start=not has_bias, stop=True)
                h_g = hpool.tile([hdim, GW], bf16, tag="h")
                nc.scalar.activation(
                    h_g[:, :gw], ps_w[:, :gw],
                    mybir.ActivationFunctionType.Relu)
                ps_v = psV.tile([odim, GW], f32, tag="psv")
                nc.tensor.matmul(
                    ps_v[:, :gw], lhsT=w2_sb[:], rhs=h_g[:, :gw],
                    start=True, stop=True)
                o_g = outp.tile([odim, GW], bf16, tag="o")
                nc.vector.scalar_tensor_tensor(
                    o_g[:, :gw], ps_v[:, :gw], 1.0, dscr_g[:, :gw],
                    mybir.AluOpType.mult, mybir.AluOpType.mult,
                )
                # out-DMA on the idle Pool/SWDGE queue (the sync engine's
                # strict FIFO would couple mexp prefetches to o_g readiness)
                nc.gpsimd.dma_start(
                    out_d[:, t0 * W:t0 * W + gw], o_g[:, :gw])

            pending = None

            for g in range(NG):
                if g + 3 < NG:
                    m_tiles.append(load_group(g + 3))
                if g + 2 < NG:
                    s_tiles.append(build_s(g + 2))
                m_t = m_tiles.pop(0)
                s_g = s_tiles.pop(0)
                gc0 = gco[g]
                t0 = int(gbounds[g])
                ntl = int(gbounds[g + 1]) - t0
                gw = ntl * W
                if layer == 1:
                    # dinv^2 scale row replicated across partitions on
                    # device: rank-1 ones-matmul + scalar copy (compute
                    # engines cannot broadcast across partitions, and DMA
                    # replication would cost 1.6MB of HBM traffic)
                    ps_r = psR.tile([odim, GW], f32, tag="psr")
                    nc.tensor.matmul(
                        ps_r[:, :gw], lhsT=ones_sb[:],
                        rhs=dsc_sb[:, t0 * W:t0 * W + gw],
                        start=True, stop=True)
                    dscr_g = gpool.tile([odim, GW], bf16, tag="dscr")
                    nc.scalar.activation(
                        dscr_g[:, :gw], ps_r[:, :gw],
                        mybir.ActivationFunctionType.Copy)
                    agg_g = aggp.tile([fdim, GROUP_TILES, W], bf16, tag="agg")
                else:
                    og_g = outp.tile([W, GROUP_TILES, odim], f32, tag="og")
                for ti in range(ntl):
                    tl = t0 + ti
                    nch = int(k_tl[tl])
                    mb = int(chunk_off[tl]) - gc0
                    if layer == 1:
                        ps_a = psA.tile([fdim, W], f32, tag="psa")
                        for ci in range(nch):
                            nc.tensor.matmul(
                                ps_a[:],
                                lhsT=m_t[:, mb + ci, :],
                                rhs=s_g[:, mb + ci, :],
                                start=(ci == 0), stop=(ci == nch - 1),
                            )
                        nc.scalar.activation(
                            agg_g[:, ti, :], ps_a[:],
                            mybir.ActivationFunctionType.Copy)
                    else:
                        # transposed agg psum [dst, vfeat]
                        ps_a = psA.tile([W, odim], f32, tag="psa")
                        first_mm = True
                        if has_bias:
                            nc.tensor.matmul(
                                ps_a[:],
                                lhsT=rdinv_sb[:, tl * W:(tl + 1) * W],
                                rhs=b_sb[:], start=True, stop=False)
                            first_mm = False
                        for ci in range(nch):
                            nc.tensor.matmul(
                                ps_a[:],
                                lhsT=s_g[:, mb + ci, :],
                                rhs=m_t[:, mb + ci, :],
                                start=(first_mm and ci == 0),
                                stop=(ci == nch - 1),
                            )
                        # per-partition dinv scale rides the psum copy
                        nc.scalar.activation(
                            og_g[:, ti, :], ps_a[:],
                            mybir.ActivationFunctionType.Copy,
                            scale=dinvt_sb[:, tl:tl + 1])
                if layer == 1:
                    # batched second GEMM tail (transposed): psW[o, (t,d)] =
                    #   W1.T @ agg + b ⊗ rdinv; relu on the scalar engine;
                    #   transform-first tail v = W2.T @ h; the dinv^2 scale
                    #   commutes through W2's column structure and is fused
                    #   into the final DVE copy of ps_v.  The tail is
                    #   DEFERRED one group: the PE executes matmuls in
                    #   program order, so emitting the tail here would stall
                    #   the PE on this group's scalar copies instead of
                    #   letting it start the next group's agg matmuls.
                    if pending is not None:
                        emit_tail(*pending)
                    pending = (t0, ntl, gw, agg_g, dscr_g)
                else:
                    # layer 2's only tail is the out-DMA, already decoupled
                    # on the Pool queue
                    nc.gpsimd.dma_start(
                        out_d[:, t0 * odim:(t0 + ntl) * odim],
                        og_g[:, :ntl, :])
            if layer == 1 and pending is not None:
                emit_tail(*pending)

    nc.compile()
    _CACHED[key] = nc
    return nc


# ================================================================== kernel
def _run_layer(layer, table, weights, b, prep, trace):
    from concourse.bass_utils import run_bass_kernel_spmd

    fdim = table.shape[1]
    has_bias = bool(np.any(np.asarray(b)))
    nc = _build_layer_nc(layer, prep["geom"], has_bias)
    base = {
        "imat": np.ascontiguousarray(prep["imat"]),
        "b": np.ascontiguousarray(np.asarray(b, np.float32).astype(BF16)[None, :]),
    }
    if layer == 1:
        base["w"] = np.ascontiguousarray(
            np.asarray(weights[0], np.float32).astype(BF16))
        base["w2"] = np.ascontiguousarray(
            np.asarray(weights[1], np.float32).astype(BF16))
    in_maps = []
    for c in range(N_CORES):
        m = dict(base)
        m["mexp"] = np.ascontiguousarray(
            _expand(table, prep["srcrows"][c], fdim)
            .reshape(P, -1))
        m["dstp"] = np.ascontiguousarray(prep["dstp"][c])
        if layer == 1:
            m["dsc"] = np.ascontiguousarray(prep["dinv2_row"][c].astype(BF16))
        else:
            m["dinvt"] = np.ascontiguousarray(
                prep["dinv_row"][c].reshape(TPC, W).T.astype(np.float32))
        m["rdinv"] = np.ascontiguousarray(prep["rdinv_row"][c])
        in_maps.append(m)
    res = run_bass_kernel_spmd(nc, in_maps, core_ids=list(range(N_CORES)),
                               trace=trace)
    full = np.concatenate(
        [_unpermute(r["out"], OUT_DIM, layer) for r in res.results], axis=0)
    return res, full


def kernel(x, edge_index, W1, b1, W2, b2):
    prep = _preprocess(x, edge_index)
    trace = bool(os.environ.get("GCN_TRACE"))

    res1, vfull = _run_layer(1, prep["xt"], (W1, W2), b1, prep, trace)
    res2, big = _run_layer(2, vfull.astype(BF16), None, b2, prep, trace)

    global LAST_RESULTS
    LAST_RESULTS = (res1, res2)
    return np.ascontiguousarray(big[prep["slot_of_node"]]).astype(np.float32)


# revision 79
# speedup vs baseline: 1.2002x; 1.2002x over previous
"""GCN encoder (2-layer GCNConv) on 8 Trainium2 NeuronCores.

Strategy (pull model, dst-sharded, host-routed halo):
  out = A @ relu(A @ x @ W1 + b1) @ W2 + b2,  A = D^-1/2 (Adj+I) D^-1/2
Reassociate: agg = A @ x first, then dense matmul by W (A@(xW) == (A@x)W).
Fold the src-side dinv into the node table on the host (x~ = dinv * x) and
the dst-side dinv into a per-partition ACT scale.

The per-edge gather (the SWDGE descriptor-generation bottleneck of the
first version: GpSimd was 90% busy emitting one descriptor per edge) is
done ON THE HOST: the edge list is known at preprocessing time and the
node table passes through the host anyway (x is an input; h1 must make a
host roundtrip because the axon terminal cannot run collectives), so the
host materializes each core's edge messages x~[src[e]] directly in the
PE-chunk layout.  The device then just streams contiguous bf16 blocks
(one fat descriptor per partition per group DMA, spread over all 16 DMA
engines) — pure HBM bandwidth, no gather.

Aggregation uses 64-wide dst tiles: chunks of 128 edges feed PE
selection-matrix matmuls (self loops folded in as ordinary edges):
  psum[feat128, dst64] += M_chunk[e, feat].T @ S_chunk[e, dst64]
The 64-wide tiles halve the S-build work on DVE (the is_equal runs at
1 elem/lane/cycle due to the broadcast dstp operand) and shorten each
matmul; S is built once per group of 7 tiles in a single batched
is_equal.  Outputs are written in a [64, tiles*odim] partition-major
DRAM layout (one descriptor per partition) and unpermuted on the host.
"""

import os

import numpy as np
import ml_dtypes

# ---------------------------------------------------------------- constants
N_NODES = 100000
N_EDGES = 1600000
IN_DIM = 128
HID_DIM = 128
OUT_DIM = 64
P = 128                     # edge-chunk size (PE contraction dim)
W = 64                      # dst-tile width

N_CORES = 8
TPC = 196                   # tiles per core
SPC = TPC * W               # 12544 slots per core
NS = N_CORES * SPC          # 100352 slots total
NT = N_CORES * TPC          # 1568 tiles total
GROUP_TILES = 7
N_GROUPS = TPC // GROUP_TILES   # 28

BF16 = ml_dtypes.bfloat16
SENT_ROW = NS               # table_ext[NS] is an all-zero row
SENT_DST = 200.0            # dst-local sentinel: matches no iota value

LAST_RESULTS = None
DEFER_TAIL = bool(int(os.environ.get("GCN_DEFER_TAIL", "1")))


# ================================================================ host prep
def _preprocess(x, edge_index):
    x = np.asarray(x, dtype=np.float32)
    ei = np.asarray(edge_index, dtype=np.int64)
    src = ei[0]
    dst = ei[1]

    # degree includes the self loop (appended by the reference)
    deg = (np.bincount(dst, minlength=N_NODES) + 1).astype(np.float64)
    dinv = 1.0 / np.sqrt(np.maximum(deg, 1e-12))

    # ---- slot assignment: band packing.  Tiles 0..N9-1 target 9 chunks
    # (<=1152 edge rows incl self), the rest 8 (<=1024), so chunk counts
    # hug ceil(rows/128) with ~0 padding instead of the ~6% a uniform
    # degree spread gives.  Nodes are snake-dealt to cores, LPT-split into
    # the two bands per core, and snake-dealt within each band.
    rows_of = deg.astype(np.int64)            # deg+1 per node... deg incl self
    order = np.argsort(-rows_of, kind="stable")
    idx = np.arange(N_NODES)
    r8 = idx // N_CORES
    p8 = idx % N_CORES
    core_of_rank = np.where(r8 % 2 == 0, p8, N_CORES - 1 - p8)

    N9 = 104                                  # tiles in the 9-chunk band
    N8 = TPC - N9
    T9_target = N9 * (9 * P - 5)              # pace 1147: slack vs both caps
    slot_of_node = np.empty(N_NODES, dtype=np.int64)
    for c in range(N_CORES):
        nodes = order[core_of_rank == c]      # this core's nodes, deg desc
        rows = rows_of[nodes]
        npad = SPC - len(nodes)               # empty slots: 1 self row each
        rows_all = np.concatenate([rows, np.ones(npad, dtype=np.int64)])
        total = int(rows_all.sum())
        # LPT split into band9 / band8 by remaining pace
        band = np.empty(len(rows_all), dtype=np.int8)
        rem9, s9 = float(min(T9_target, total - N8 * W)), N9 * W
        rem8, s8 = float(total) - rem9, N8 * W
        for i, rv in enumerate(rows_all):
            if s9 > 0 and (s8 == 0 or rem9 * s8 >= rem8 * s9):
                band[i] = 0
                rem9 -= rv
                s9 -= 1
            else:
                band[i] = 1
                rem8 -= rv
                s8 -= 1
        # snake within each band over its tiles
        for bid, t0, ntl in ((0, 0, N9), (1, N9, N8)):
            sel = np.nonzero(band == bid)[0]
            sel = sel[sel < len(nodes)]       # drop empty-slot dummies
            kk = np.arange(len(sel))
            rr = kk // ntl
            pp = kk % ntl
            tl = t0 + np.where(rr % 2 == 0, pp, ntl - 1 - pp)
            slot_of_node[nodes[sel]] = (c * TPC + tl) * W + rr

    # ---- per-edge quantities (self loops folded in as ordinary edges)
    all_dst = np.concatenate([slot_of_node[dst], np.arange(NS, dtype=np.int64)])
    all_src = np.concatenate([slot_of_node[src], np.arange(NS, dtype=np.int64)])
    gtile = all_dst // W                      # global tile id
    dstl = (all_dst % W).astype(np.int32)

    order_e = np.argsort(gtile, kind="stable")
    gt_sorted = gtile[order_e]
    seg = np.searchsorted(gt_sorted, np.arange(NT + 1))
    seg_len = np.diff(seg)                    # edges (incl self) per gtile

    # shared static geometry: chunks per local tile = max over cores
    per_core_len = seg_len.reshape(N_CORES, TPC)
    k_tl = (-(-per_core_len.max(axis=0) // P)).astype(np.int64)   # [TPC]
    chunk_off = np.concatenate([[0], np.cumsum(k_tl)])
    C_TOT = int(chunk_off[-1])
    NCH_MAX = int(k_tl.max())
    # variable-size groups: two small warm-up groups so the pipeline
    # fills quickly, then steady 7-tile groups
    gtiles = [2, 5] + [GROUP_TILES] * ((TPC - 7) // GROUP_TILES)
    assert sum(gtiles) == TPC
    gbounds = np.concatenate([[0], np.cumsum(gtiles)])
    gco = [int(chunk_off[b]) for b in gbounds]
    NG = len(gtiles)
    C_GRP_MAX = max(gco[g + 1] - gco[g] for g in range(NG))

    # ---- per-core fill of srcrows (gather plan) and dstp (dst-local ids)
    rank = np.arange(len(order_e), dtype=np.int64) - seg[gt_sorted]
    tl_sorted = gt_sorted % TPC
    core_sorted = gt_sorted // TPC
    col_local = chunk_off[tl_sorted] + rank // P          # chunk col in core
    e_local = rank % P

    srcrows = np.full((N_CORES, C_TOT * P), SENT_ROW, dtype=np.int64)
    srcrows[core_sorted, col_local * P + e_local] = all_src[order_e]
    dstp = np.full((N_CORES, P, C_TOT), SENT_DST, dtype=BF16)
    dstp[core_sorted, e_local, col_local] = dstl[order_e].astype(BF16)



    # ---- per-core dinv (ACT scale) and rdinv (bias rank-1 lhsT)
    dinv_slots = np.zeros(NS, dtype=np.float32)
    dinv_slots[slot_of_node] = dinv.astype(np.float32)
    rdinv_slots = np.zeros(NS, dtype=np.float32)
    rdinv_slots[slot_of_node] = (1.0 / dinv).astype(np.float32)
    dinv_row = dinv_slots.reshape(N_CORES, 1, SPC)                          # [c,1,12544]
    # layer-1 scale is dinv^2: it also folds the src-side dinv the
    # layer-2 table needs into h1 (relu commutes with scale>0)
    dinv2_row = (dinv_row * dinv_row).astype(np.float32)
    rdinv_row = rdinv_slots.reshape(N_CORES, 1, SPC).astype(BF16)           # [c,1,12544]

    # imat[e, c, d] = d  (constant is_equal operand, batched per group)
    imat = np.broadcast_to(
        np.arange(W, dtype=np.float32).astype(BF16)[None, None, :],
        (P, C_GRP_MAX, W)).reshape(P, C_GRP_MAX * W).copy()

    # ---- node-feature table in slot order, pre-scaled by dinv (bf16)
    xt = np.zeros((NS, IN_DIM), dtype=BF16)
    xt[slot_of_node] = (x * dinv[:, None].astype(np.float32)).astype(BF16)

    geom = dict(
        k_tl=k_tl, chunk_off=chunk_off, C_TOT=C_TOT,
        NCH_MAX=NCH_MAX, gco=gco, C_GRP_MAX=C_GRP_MAX,
        gbounds=gbounds, NG=NG,
    )
    return dict(
        srcrows=srcrows, dstp=dstp, xt=xt, imat=imat,
        dinv_row=dinv_row, dinv2_row=dinv2_row, rdinv_row=rdinv_row,
        slot_of_node=slot_of_node, geom=geom,
    )


def _expand(table, srcrows_c, fdim):
    """Host-side halo routing: materialize per-edge messages in PE-chunk
    layout [128 e, C_TOT, fdim] bf16 from the slot table (+ zero pad row)."""
    table_ext = np.vstack([table, np.zeros((1, fdim), dtype=table.dtype)])
    rows = table_ext[srcrows_c]                       # [C_TOT*128, fdim]
    ctot = rows.shape[0] // P
    return np.ascontiguousarray(
        rows.reshape(ctot, P, fdim).transpose(1, 0, 2))


def _unpermute(out_c, odim, layer):
    """Device layout -> [SPC, odim] slot-major."""
    if layer == 1:                            # [odim, SPC]
        return np.ascontiguousarray(out_c.T)
    # layer 2: [W, TPC*odim]
    return np.ascontiguousarray(
        out_c.reshape(W, TPC, odim).transpose(1, 0, 2).reshape(SPC, odim))


# ============================================================ numpy emulator
def _emulate(prep, W1, b1, W2, b2):
    """Fast numpy mirror of the device kernel (fp32 math on bf16-rounded
    data) to validate the host-side layout before burning a HW run."""
    geom = prep["geom"]
    C_TOT = geom["C_TOT"]
    k_tl = geom["k_tl"]
    chunk_off = geom["chunk_off"]
    rdinv = prep["rdinv_row"].astype(np.float32)

    # dst slot (core-local) of every mexp position, sentinel -1
    dst_of_pos = np.full((N_CORES, C_TOT * P), -1, dtype=np.int64)
    for c in range(N_CORES):
        d = prep["dstp"][c].astype(np.float32)        # [128, C_TOT]
        for tl in range(TPC):
            for j in range(int(k_tl[tl])):
                col = chunk_off[tl] + j
                dloc = d[:, col]
                valid = dloc < W
                dst_of_pos[c, (col * P + np.arange(P))[valid]] = \
                    tl * W + dloc[valid].astype(np.int64)

    def layer(table, w, bvec, relu, odim, scale_t, sc_bf16=True):
        out = np.zeros((N_CORES, SPC, table.shape[1]), dtype=np.float32)
        for c in range(N_CORES):
            rows = _expand(table, prep["srcrows"][c], table.shape[1])
            rows = rows.transpose(1, 0, 2).reshape(C_TOT * P, -1).astype(np.float32)
            dpos = dst_of_pos[c]
            valid = dpos >= 0
            o = np.argsort(dpos[valid], kind="stable")
            rv = rows[valid][o]
            dv = dpos[valid][o]
            agg = np.zeros((SPC + 1, table.shape[1]), dtype=np.float32)
            uniq = np.unique(dv)
            red = np.add.reduceat(rv, np.searchsorted(dv, uniq), axis=0)
            agg[uniq] = red
            out[c] = agg[:SPC]
        res = np.zeros((N_CORES, SPC, odim), dtype=np.float32)
        for c in range(N_CORES):
            aggT = out[c].astype(BF16).astype(np.float32)
            ps = aggT if w is None else aggT @ w
            ps = ps + rdinv[c, 0][:, None] * bvec[None, :]
            if relu:
                ps = np.maximum(ps, 0.0)
            sc = scale_t[c, 0]
            if sc_bf16:
                sc = sc.astype(BF16)
            res[c] = ps * sc.astype(np.float32)[:, None]
        return res

    w1 = np.asarray(W1, np.float32).astype(BF16).astype(np.float32)
    w2 = np.asarray(W2, np.float32).astype(BF16).astype(np.float32)
    b1f = np.asarray(b1, np.float32).astype(BF16).astype(np.float32)
    b2f = np.asarray(b2, np.float32).astype(BF16).astype(np.float32)
    h1 = layer(prep["xt"], w1, b1f, True, HID_DIM, prep["dinv2_row"])
    h1t = h1.reshape(NS, HID_DIM).astype(BF16).astype(np.float32)
    # transform-first: v = h1~ @ W2 happens at the tail of the layer-1 NEFF
    vt = (h1t @ w2).astype(BF16)
    out = layer(vt, None, b2f, False, OUT_DIM, prep["dinv_row"], sc_bf16=False)
    return out.reshape(NS, OUT_DIM)[prep["slot_of_node"]]


# ============================================================= bass kernel
# The axon terminal cannot run ncfw collectives (NRT_EXEC_UNIT_UNRECOVERABLE),
# so the two GCN layers run as two NEFFs with a host-side h1 exchange; the
# host also routes the per-edge halo (expanded message tables) for each NEFF.
_CACHED = {}


def _build_layer_nc(layer, geom, has_bias):
    gkey = (geom["C_TOT"], hash(geom["k_tl"].tobytes()))
    key = (layer, gkey, has_bias)
    if key in _CACHED:
        return _CACHED[key]

    import concourse.mybir as mybir
    import concourse.tile as tile
    from concourse import bacc

    f32 = mybir.dt.float32
    bf16 = mybir.dt.bfloat16

    # layer 1: 128-wide messages, psW = W1.T@agg (+bias), relu, x dinv^2,
    #          then transform-first tail v = (.)@W2 -> bf16 [64, SPC] out.
    # layer 2: 64-wide pre-transformed messages, agg IS the output
    #          (+rank-1 bias in psum), x dinv -> f32 [64, SPC] out.
    fdim = IN_DIM if layer == 1 else OUT_DIM
    hdim = HID_DIM                      # layer-1 hidden width
    odim = OUT_DIM
    relu = layer == 1

    nc = bacc.Bacc("TRN2", target_bir_lowering=False, debug=False,
                   num_devices=N_CORES, name=f"gcnx_l{layer}")

    k_tl = geom["k_tl"]
    chunk_off = geom["chunk_off"]
    C_TOT = geom["C_TOT"]
    gco = geom["gco"]
    C_GRP_MAX = geom["C_GRP_MAX"]
    gbounds = geom["gbounds"]
    NG = geom["NG"]

    GW = GROUP_TILES * W

    mexp_d = nc.dram_tensor("mexp", [P, C_TOT * fdim], bf16, kind="ExternalInput")
    dstp_d = nc.dram_tensor("dstp", [P, C_TOT], bf16, kind="ExternalInput")
    imat_d = nc.dram_tensor("imat", [P, C_GRP_MAX * W], bf16,
                            kind="ExternalInput")
    if layer == 1:
        dsc_d = nc.dram_tensor("dsc", [1, SPC], bf16, kind="ExternalInput")
        w_d = nc.dram_tensor("w", [IN_DIM, hdim], bf16, kind="ExternalInput")
        w2_d = nc.dram_tensor("w2", [hdim, odim], bf16, kind="ExternalInput")
        b_d = nc.dram_tensor("b", [1, hdim], bf16, kind="ExternalInput")
        rdinv_d = nc.dram_tensor("rdinv", [1, SPC], bf16, kind="ExternalInput")
        out_d = nc.dram_tensor("out", [odim, SPC], bf16, kind="ExternalOutput")
    else:
        # transposed agg psum [dst, vfeat]: the dinv scale rides the ACT
        # copy's per-partition scale; bias (when present) rides the psum.
        dinvt_d = nc.dram_tensor("dinvt", [W, TPC], f32, kind="ExternalInput")
        b_d = nc.dram_tensor("b", [1, odim], bf16, kind="ExternalInput")
        rdinv_d = nc.dram_tensor("rdinv", [1, SPC], bf16, kind="ExternalInput")
        out_d = nc.dram_tensor("out", [W, TPC * odim], f32, kind="ExternalOutput")

    with tile.TileContext(nc) as tc:
        with (
            tc.tile_pool(name="const", bufs=1) as constp,
            tc.tile_pool(name="mbuf", bufs=5) as mpool,
            tc.tile_pool(name="gpool", bufs=3) as gpool,
            tc.tile_pool(name="sbuf_s", bufs=4) as spool,
            tc.tile_pool(name="agg", bufs=3) as aggp,
            tc.tile_pool(name="hbuf", bufs=3) as hpool,
            tc.tile_pool(name="outp", bufs=3) as outp,
            tc.tile_pool(name="psA", bufs=4, space="PSUM") as psA,
            tc.tile_pool(name="psB", bufs=2, space="PSUM") as psB,
            tc.tile_pool(name="psV", bufs=1, space="PSUM") as psV,
            tc.tile_pool(name="psR", bufs=1, space="PSUM") as psR,
        ):
            def load_group(g):
                ncols = gco[g + 1] - gco[g]
                m_t = mpool.tile([P, C_GRP_MAX, fdim], bf16, tag="m")
                nc.sync.dma_start(
                    m_t[:, :ncols, :],
                    mexp_d[:, gco[g] * fdim:gco[g + 1] * fdim]
                    .rearrange("p (c f) -> p c f", f=fdim))
                return m_t

            # is_equal operands first so S-building starts immediately,
            # then group prefetch, then the remaining constants
            dstp_sb = constp.tile([P, C_TOT], bf16)
            nc.sync.dma_start(dstp_sb[:], dstp_d[:, :])
            imat_sb = constp.tile([P, C_GRP_MAX, W], bf16)
            nc.sync.dma_start(
                imat_sb[:],
                imat_d[:, :].rearrange("p (c d) -> p c d", d=W))
            m_tiles = [load_group(0), load_group(1), load_group(2)]
            rdinv_sb = constp.tile([1, SPC], bf16)
            nc.sync.dma_start(rdinv_sb[:], rdinv_d[:, :])
            if layer == 1:
                w_sb = constp.tile([IN_DIM, hdim], bf16)
                nc.sync.dma_start(w_sb[:], w_d[:, :])
                w2_sb = constp.tile([hdim, odim], bf16)
                nc.sync.dma_start(w2_sb[:], w2_d[:, :])
                dsc_sb = constp.tile([1, SPC], bf16)
                nc.sync.dma_start(dsc_sb[:], dsc_d[:, :])
                ones_sb = constp.tile([1, odim], bf16)
                nc.gpsimd.memset(ones_sb[:], 1.0)
                b_sb = constp.tile([1, hdim], bf16)
            else:
                dinvt_sb = constp.tile([W, TPC], f32)
                nc.sync.dma_start(dinvt_sb[:], dinvt_d[:, :])
                b_sb = constp.tile([1, odim], bf16)
            nc.sync.dma_start(b_sb[:], b_d[:, :])

            def build_s(g):
                # S[e, c, d] = (dstp[e, c] == d) for the whole group at
                # once.  PE strided operands are catastrophic in either
                # slot (rhs 227ns/MM, lhsT similar), so S must stay
                # [e, c, d]-contiguous and the is_equal runs 1x.
                gc0 = gco[g]
                ncols = gco[g + 1] - gc0
                s_g = spool.tile([P, C_GRP_MAX, W], bf16, tag="s")
                nc.vector.tensor_tensor(
                    s_g[:, :ncols, :],
                    dstp_sb[:, gc0:gc0 + ncols]
                    .unsqueeze(2).broadcast_to([P, ncols, W]),
                    imat_sb[:, :ncols, :],
                    mybir.AluOpType.is_equal,
                )
                return s_g

            # S-builds primed 2 groups ahead: the DVE is strict FIFO, so
            # the late-dependency stt of group g must not sit in front of
            # the is_equal needed by group g+2
            s_tiles = [build_s(0), build_s(1)]

            def emit_tail(t0, ntl, gw, agg_g, dscr_g):
                agg_flat = agg_g[:, :ntl, :].rearrange("p t d -> p (t d)")
                ps_w = psB.tile([hdim, GW], f32, tag="psw")
                if has_bias:
                    nc.tensor.matmul(
                        ps_w[:, :gw], lhsT=b_sb[:],
                        rhs=rdinv_sb[:, t0 * W:t0 * W + gw],
                        start=True, stop=False)
                nc.tensor.matmul(
                    ps_w[:, :gw], lhsT=w_sb[:], rhs=agg_flat,
                    start=not has_bias, stop=True)
                h_g = hpool.tile([hdim, GW], bf16, tag="h")
                nc.scalar.activation(
                    h_g[:, :gw], ps_w[:, :gw],
                    mybir.ActivationFunctionType.Relu)
                ps_v = psV.tile([odim, GW], f32, tag="psv")
                nc.tensor.matmul(
                    ps_v[:, :gw], lhsT=w2_sb[:], rhs=h_g[:, :gw],
                    start=True, stop=True)
                o_g = outp.tile([odim, GW], bf16, tag="o")
                nc.vector.scalar_tensor_tensor(
                    o_g[:, :gw], ps_v[:, :gw], 1.0, dscr_g[:, :gw],
                    mybir.AluOpType.mult, mybir.AluOpType.mult,
                )
                # out-DMA on the idle Pool/SWDGE queue (the sync engine's
                # strict FIFO would couple mexp prefetches to o_g readiness)
                nc.gpsimd.dma_start(
                    out_d[:, t0 * W:t0 * W + gw], o_g[:, :gw])

            pending = None

            for g in range(NG):
                if g + 3 < NG:
                    m_tiles.append(load_group(g + 3))
                if g + 2 < NG:
                    s_tiles.append(build_s(g + 2))
                m_t = m_tiles.pop(0)
                s_g = s_tiles.pop(0)
                gc0 = gco[g]
                t0 = int(gbounds[g])
                ntl = int(gbounds[g + 1]) - t0
                gw = ntl * W
                if layer == 1:
                    # dinv^2 scale row replicated across partitions on
                    # device: rank-1 ones-matmul + scalar copy (compute
                    # engines cannot broadcast across partitions, and DMA
                    # replication would cost 1.6MB of HBM traffic)
                    ps_r = psR.tile([odim, GW], f32, tag="psr")
                    nc.tensor.matmul(
                        ps_r[:, :gw], lhsT=ones_sb[:],
                        rhs=dsc_sb[:, t0 * W:t0 * W + gw],
                        start=True, stop=True)
                    dscr_g = gpool.tile([odim, GW], bf16, tag="dscr")
                    nc.scalar.activation(
                        dscr_g[:, :gw], ps_r[:, :gw],
                        mybir.ActivationFunctionType.Copy)
                    agg_g = aggp.tile([fdim, GROUP_TILES, W], bf16, tag="agg")
                else:
                    og_g = outp.tile([W, GROUP_TILES, odim], f32, tag="og")
                for ti in range(ntl):
                    tl = t0 + ti
                    nch = int(k_tl[tl])
                    mb = int(chunk_off[tl]) - gc0
                    if layer == 1:
                        ps_a = psA.tile([fdim, W], f32, tag="psa")
                        for ci in range(nch):
                            nc.tensor.matmul(
                                ps_a[:],
                                lhsT=m_t[:, mb + ci, :],
                                rhs=s_g[:, mb + ci, :],
                                start=(ci == 0), stop=(ci == nch - 1),
                            )
                        nc.scalar.activation(
                            agg_g[:, ti, :], ps_a[:],
                            mybir.ActivationFunctionType.Copy)
                    else:
                        # transposed agg psum [dst, vfeat]
                        ps_a = psA.tile([W, odim], f32, tag="psa")
                        first_mm = True
                        if has_bias:
                            nc.tensor.matmul(
                                ps_a[:],
                                lhsT=rdinv_sb[:, tl * W:(tl + 1) * W],
                                rhs=b_sb[:], start=True, stop=False)
                            first_mm = False
                        for ci in range(nch):
                            nc.tensor.matmul(
                                ps_a[:],
                                lhsT=s_g[:, mb + ci, :],
                                rhs=m_t[:, mb + ci, :],
                                start=(first_mm and ci == 0),
                                stop=(ci == nch - 1),
                            )
                        # per-partition dinv scale rides the psum copy
                        nc.scalar.activation(
                            og_g[:, ti, :], ps_a[:],
                            mybir.ActivationFunctionType.Copy,
                            scale=dinvt_sb[:, tl:tl + 1])
                if layer == 1:
                    # batched second GEMM tail (transposed): psW[o, (t,d)] =
                    #   W1.T @ agg + b ⊗ rdinv; relu on the scalar engine;
                    #   transform-first tail v = W2.T @ h; the dinv^2 scale
                    #   commutes through W2's column structure and is fused
                    #   into the final DVE copy of ps_v.  The tail is
                    #   DEFERRED one group: the PE executes matmuls in
                    #   program order, so emitting the tail here would stall
                    #   the PE on this group's scalar copies instead of
                    #   letting it start the next group's agg matmuls.
                    if DEFER_TAIL:
                        if pending is not None:
                            emit_tail(*pending)
                        pending = (t0, ntl, gw, agg_g, dscr_g)
                    else:
                        emit_tail(t0, ntl, gw, agg_g, dscr_g)
                else:
                    # layer 2's only tail is the out-DMA, already decoupled
                    # on the Pool queue
                    nc.gpsimd.dma_start(
                        out_d[:, t0 * odim:(t0 + ntl) * odim],
                        og_g[:, :ntl, :])
            if layer == 1 and pending is not None:
                emit_tail(*pending)

    nc.compile()
    _CACHED[key] = nc
    return nc


# ================================================================== kernel
def _run_layer(layer, table, weights, b, prep, trace):
    from concourse.bass_utils import run_bass_kernel_spmd

    fdim = table.shape[1]
    has_bias = bool(np.any(np.asarray(b)))
    nc = _build_layer_nc(layer, prep["geom"], has_bias)
    base = {
        "imat": np.ascontiguousarray(prep["imat"]),
        "b": np.ascontiguousarray(np.asarray(b, np.float32).astype(BF16)[None, :]),
    }
    if layer == 1:
        base["w"] = np.ascontiguousarray(
            np.asarray(weights[0], np.float32).astype(BF16))
        base["w2"] = np.ascontiguousarray(
            np.asarray(weights[1], np.float32).astype(BF16))
    in_maps = []
    for c in range(N_CORES):
        m = dict(base)
        m["mexp"] = np.ascontiguousarray(
            _expand(table, prep["srcrows"][c], fdim)
            .reshape(P, -1))
        m["dstp"] = np.ascontiguousarray(prep["dstp"][c])
        if layer == 1:
            m["dsc"] = np.ascontiguousarray(prep["dinv2_row"][c].astype(BF16))
        else:
            m["dinvt"] = np.ascontiguousarray(
                prep["dinv_row"][c].reshape(TPC, W).T.astype(np.float32))
        m["rdinv"] = np.ascontiguousarray(prep["rdinv_row"][c])
        in_maps.append(m)
    res = run_bass_kernel_spmd(nc, in_maps, core_ids=list(range(N_CORES)),
                               trace=trace)
    full = np.concatenate(
        [_unpermute(r["out"], OUT_DIM, layer) for r in res.results], axis=0)
    return res, full


def kernel(x, edge_index, W1, b1, W2, b2):
    prep = _preprocess(x, edge_index)
    trace = bool(os.environ.get("GCN_TRACE"))

    res1, vfull = _run_layer(1, prep["xt"], (W1, W2), b1, prep, trace)
    res2, big = _run_layer(2, vfull.astype(BF16), None, b2, prep, trace)

    global LAST_RESULTS
    LAST_RESULTS = (res1, res2)
    return np.ascontiguousarray(big[prep["slot_of_node"]]).astype(np.float32)
